# revision 9
# baseline (speedup 1.0000x reference)
"""Trainium2 Bass kernel for nn_Half_Graph (GNN message passing half-graph).

Pure data-parallel over batch B=8 -> 8 NeuronCores, one sample per core.
Weights replicated; per-sample tensors sharded by batch index.

Outputs match reference: (stack([n0,n1,n2]) [3,8,10,96,96], dm [8,2,96,96]).

Key constraints honored:
- compute-engine SBUF operands must start at partition 0/32/64/96
- DMA access patterns: max 3 dims; DRAM-side APs may have 0-strides
  (used for partition broadcasts); SBUF-side may start anywhere
- fp32 matmuls run at full rate as float32r when N >= 256
"""

import numpy as np

import concourse.bass as bass
import concourse.tile as tile
from concourse import bacc, mybir
from concourse.bass_utils import run_bass_kernel_spmd

F32 = mybir.dt.float32
F32R = mybir.dt.float32r

N_CORES = 8
B = 8
HID = 10
H = W = 96
HP = 98  # padded
NPIX = H * W  # 9216
NPPIX = HP * HP  # 9604

ROWS_PER_TILE = 4
N_TILES = H // ROWS_PER_TILE  # 24
NT = ROWS_PER_TILE * W  # 384 columns per tile

# pad_all row map ([106, 9604] padded images for branch convs).
# Compute-written blocks at aligned bases; DMA-written blocks anywhere;
# gap rows stay zero (memset once) and have zero lhsT rows.
P_P0 = 0  # parent * m0            (DVE write, base 0)
P_H1 = 10  # h_nodes[1]            (DMA)
P_P1 = 32  # parent * m1           (DVE write, base 32)
P_H2 = 42  # h_nodes[2]            (DMA)
P_UP = 64  # up composition        (DMA, host-precomputed)
P_LO = 96  # lo composition        (DMA, host-precomputed)
P_ROWS = 106


def _taps():
    return [(ty, tx) for ty in range(3) for tx in range(3)]


# --------------------------------------------------------------------------
# Host-side packing
# --------------------------------------------------------------------------


def pack_weights(inp):
    w = {}
    f32 = np.float32

    # --- big conv (decomp_att conv1) ---
    W1f = (inp["da_w1"] * inp["da_g1"][:, None, None, None]).astype(f32)  # (256,257,3,3)
    Wbig = np.zeros((128, 9, 2, 2, 128), f32)  # [ci, tap, cih, coh, co]
    for t, (ty, tx) in enumerate(_taps()):
        for ih in range(2):
            for oh in range(2):
                Wbig[:, t, ih, oh, :] = W1f[
                    oh * 128 : (oh + 1) * 128, 1 + ih * 128 : 1 + (ih + 1) * 128, ty, tx
                ].T
    w["Wbig"] = np.ascontiguousarray(Wbig.reshape(128, 9 * 2 * 2 * 128))
    Watt = np.zeros((9, 256), f32)
    for t, (ty, tx) in enumerate(_taps()):
        Watt[t, :] = W1f[:, 0, ty, tx]
    w["Watt"] = Watt
    w["bias_big"] = np.stack([inp["da_b1"][:128], inp["da_b1"][128:]], axis=1).astype(f32)

    # --- dm conv (1x1 256->2) + sdiff column ---
    W2f = inp["da_w2"][:, :, 0, 0].astype(f32)  # (2,256)
    # cols: 0=dm0, 1=dm1, 32=dm0-dm1 (PSUM base-32 aligned), rest zero
    W2e = np.zeros((128, 2, 33), f32)
    for ih in range(2):
        W2e[:, ih, 0] = W2f[0, ih * 128 : (ih + 1) * 128]
        W2e[:, ih, 1] = W2f[1, ih * 128 : (ih + 1) * 128]
        W2e[:, ih, 32] = W2e[:, ih, 0] - W2e[:, ih, 1]
    w["W2e"] = np.ascontiguousarray(W2e.reshape(128, 66))
    w["bias_dm"] = inp["da_b2"].reshape(2, 1).astype(f32)
    w["bias_sd"] = np.array([[inp["da_b2"][0] - inp["da_b2"][1]]], f32)

    # --- branch 3x3 convs (block-sparse over pad_all rows) ---
    relW1 = (inp["rel_w1"] * inp["rel_g1"][:, None, None, None]).astype(f32)
    cuW1 = (inp["cu_w1"] * inp["cu_g1"][:, None, None, None]).astype(f32)
    clW1 = (inp["cl_w1"] * inp["cl_g1"][:, None, None, None]).astype(f32)
    L3 = np.zeros((P_ROWS, 9, 40), f32)
    for t, (ty, tx) in enumerate(_taps()):
        # out cols: d0 0-9, d1 10-19, cu 20-29, cl 30-39
        L3[P_P0 : P_P0 + 10, t, 0:10] = relW1[:, 0:10, ty, tx].T
        L3[P_H1 : P_H1 + 10, t, 0:10] = relW1[:, 10:20, ty, tx].T
        L3[P_P1 : P_P1 + 10, t, 10:20] = relW1[:, 0:10, ty, tx].T
        L3[P_H2 : P_H2 + 10, t, 10:20] = relW1[:, 10:20, ty, tx].T
        L3[P_H1 : P_H1 + 10, t, 20:30] = cuW1[:, 0:10, ty, tx].T
        L3[P_UP : P_UP + 10, t, 20:30] = cuW1[:, 10:20, ty, tx].T
        L3[P_H2 : P_H2 + 10, t, 30:40] = clW1[:, 0:10, ty, tx].T
        L3[P_LO : P_LO + 10, t, 30:40] = clW1[:, 10:20, ty, tx].T
    w["L3"] = np.ascontiguousarray(L3.reshape(P_ROWS, 9 * 40))
    w["bias3"] = np.concatenate(
        [inp["rel_b1"], inp["rel_b1"], inp["cu_b1"], inp["cl_b1"]]
    ).reshape(40, 1).astype(f32)

    # --- branch 1x1 convs: psum rows d0 d1 cu cl ---
    relW2 = (inp["rel_w2"][:, :, 0, 0] * inp["rel_g2"][:, None]).astype(f32)
    cuW2 = (inp["cu_w2"][:, :, 0, 0] * inp["cu_g2"][:, None]).astype(f32)
    clW2 = (inp["cl_w2"][:, :, 0, 0] * inp["cl_g2"][:, None]).astype(f32)
    # out cols: d0 0-9, d1 10-19, cu 32-41, cl 42-51 (PSUM base-32 aligned)
    L1 = np.zeros((40, 52), f32)
    L1[0:10, 0:10] = relW2.T
    L1[10:20, 10:20] = relW2.T
    L1[20:30, 32:42] = cuW2.T
    L1[30:40, 42:52] = clW2.T
    w["L1"] = L1
    w["bias1x1_d"] = np.concatenate([inp["rel_b2"], inp["rel_b2"]]).reshape(20, 1).astype(f32)
    w["bias1x1_c"] = np.concatenate([inp["cu_b2"], inp["cl_b2"]]).reshape(20, 1).astype(f32)

    # --- GRU gates, split-K: gates = Lg_x0.T x0 + Lg_x12.T x12 + Lg_h.T h ---
    # psum rows: r0 r1 r2 u0 u1 u2
    Lg_x0 = np.zeros((10, 6), f32)
    Lg_x12 = np.zeros((20, 6), f32)
    Lg_h = np.zeros((30, 6), f32)
    for k in range(3):
        gw = inp["gru_gw"][k][:, :, 0, 0].astype(f32)  # (2, 20)
        if k == 0:
            Lg_x0[:, 0] = gw[0, 0:10]
            Lg_x0[:, 3] = gw[1, 0:10]
        else:
            Lg_x12[(k - 1) * 10 : k * 10, k] = gw[0, 0:10]
            Lg_x12[(k - 1) * 10 : k * 10, 3 + k] = gw[1, 0:10]
        Lg_h[k * 10 : (k + 1) * 10, k] = gw[0, 10:20]
        Lg_h[k * 10 : (k + 1) * 10, 3 + k] = gw[1, 10:20]
    w["Lg_x0"] = Lg_x0
    w["Lg_x12"] = Lg_x12
    w["Lg_h"] = Lg_h
    w["bias_g"] = np.concatenate(
        [inp["gru_gb"][:, 0], inp["gru_gb"][:, 1]]
    ).reshape(6, 1).astype(f32)

    # --- GRU cnm, split-K ---
    Lc_x0 = np.zeros((10, 30), f32)
    Lc_x12 = np.zeros((20, 30), f32)
    Lc_rh = np.zeros((30, 30), f32)
    for k in range(3):
        cw = (inp["gru_cw"][k][:, :, 0, 0] * inp["gru_cg"][k][:, None]).astype(f32)  # (10,20)
        if k == 0:
            Lc_x0[:, 0:10] = cw[:, 0:10].T
        else:
            Lc_x12[(k - 1) * 10 : k * 10, k * 10 : (k + 1) * 10] = cw[:, 0:10].T
        Lc_rh[k * 10 : (k + 1) * 10, k * 10 : (k + 1) * 10] = cw[:, 10:20].T
    w["Lc_x0"] = Lc_x0
    w["Lc_x12"] = Lc_x12
    w["Lc_rh"] = Lc_rh
    w["bias_c"] = inp["gru_cb"].reshape(30, 1).astype(f32)
    return w


def pack_percore(inp, b):
    f32 = np.float32
    d = {}
    # padded xh: [ci(128), cih(2), 98, 98]
    xh = inp["xh"][b].reshape(2, 128, 96, 96)
    xp = np.zeros((2, 128, HP, HP), f32)
    xp[:, :, 1:97, 1:97] = xh
    d["xh_pad"] = np.ascontiguousarray(xp.transpose(1, 0, 2, 3).reshape(128, 2 * NPPIX))
    # shifted parent-attention copies (one per conv tap)
    ap = np.zeros((HP, HP), f32)
    ap[1:97, 1:97] = inp["f_atts"][1, b, 0]
    ash = np.empty((9, NPIX), f32)
    for t, (ty, tx) in enumerate(_taps()):
        ash[t] = ap[ty : ty + 96, tx : tx + 96].reshape(-1)
    d["att_shift"] = ash
    d["att_img"] = np.ascontiguousarray(inp["f_atts"][1, b, 0].reshape(1, NPIX))
    # input-only sums / products (host precompute)
    d["x0"] = (inp["f_nodes"][0, b] + inp["p_nodes"][0, b]).reshape(10, NPIX).astype(f32)
    d["upP"] = (
        inp["p_nodes"][1:5, b].sum(0) * inp["p_atts"][1:5, b].sum(0)
    ).reshape(10, NPIX).astype(f32)
    d["loP"] = (
        inp["p_nodes"][5:7, b].sum(0) * inp["p_atts"][5:7, b].sum(0)
    ).reshape(10, NPIX).astype(f32)
    d["parent"] = np.ascontiguousarray(inp["f_nodes"][1, b].reshape(10, NPIX))
    d["h_nodes"] = np.ascontiguousarray(inp["h_nodes"][:, b].reshape(30, NPIX))
    return d


# --------------------------------------------------------------------------
# Device kernel
# --------------------------------------------------------------------------

# tensors feeding matmuls are declared float32r end-to-end (same bits as f32)
R_NAMES = {
    "Wbig", "Watt", "W2e", "L3", "L1", "Lg_x0", "Lg_x12", "Lg_h",
    "Lc_x0", "Lc_x12", "Lc_rh", "xh_pad", "att_shift", "x0", "upP",
    "loP", "h_nodes",
}

WEIGHT_SPECS = [
    ("Wbig", (128, 4608)),
    ("Watt", (9, 256)),
    ("bias_big", (128, 2)),
    ("W2e", (128, 66)),
    ("bias_dm", (2, 1)),
    ("bias_sd", (1, 1)),
    ("L3", (P_ROWS, 360)),
    ("bias3", (40, 1)),
    ("L1", (40, 52)),
    ("bias1x1_d", (20, 1)),
    ("bias1x1_c", (20, 1)),
    ("Lg_x0", (10, 6)),
    ("Lg_x12", (20, 6)),
    ("Lg_h", (30, 6)),
    ("bias_g", (6, 1)),
    ("Lc_x0", (10, 30)),
    ("Lc_x12", (20, 30)),
    ("Lc_rh", (30, 30)),
    ("bias_c", (30, 1)),
]

PERCORE_SPECS = [
    ("xh_pad", (128, 2 * NPPIX)),
    ("att_shift", (9, NPIX)),
    ("att_img", (1, NPIX)),
    ("x0", (10, NPIX)),
    ("upP", (10, NPIX)),
    ("loP", (10, NPIX)),
    ("parent", (10, NPIX)),
    ("h_nodes", (30, NPIX)),
]


def dt_of(name):
    return F32R if name in R_NAMES else F32


def r32(ap):
    return ap.bitcast(F32R)


def build_nc():
    nc = bacc.Bacc("TRN2", target_bir_lowering=False, debug=False, num_devices=N_CORES)

    ins = {}
    for name, shape in WEIGHT_SPECS + PERCORE_SPECS:
        ins[name] = nc.dram_tensor(name, list(shape), dt_of(name), kind="ExternalInput").ap()
    out_nodes = nc.dram_tensor("out_nodes", [30, NPIX], F32, kind="ExternalOutput").ap()
    out_dm = nc.dram_tensor("out_dm", [2, NPIX], F32, kind="ExternalOutput").ap()

    RELU = mybir.ActivationFunctionType.Relu
    SIG = mybir.ActivationFunctionType.Sigmoid

    def tiles():
        for it in range(N_TILES):
            y0 = it * ROWS_PER_TILE
            yield it, y0, slice(y0 * W, (y0 + ROWS_PER_TILE) * W)

    with tile.TileContext(nc) as tc:
        with (
            tc.tile_pool(name="wpool", bufs=1) as wpool,
            tc.tile_pool(name="drampool", bufs=1, space="DRAM") as drampool,
        ):
            wt = {}
            for name, shape in WEIGHT_SPECS:
                wt[name] = wpool.tile(list(shape), dt_of(name), tag=name, name="wt_" + name)
                nc.sync.dma_start(wt[name][:], ins[name][:])

            # DRAM bounce buffers (for partition broadcasts)
            m0_d = drampool.tile([1, NPIX], F32, name="m0_d")
            m1_d = drampool.tile([1, NPIX], F32, name="m1_d")
            ru_d = drampool.tile([6, NPIX], F32, name="ru_d")

            # ---------------- Phase 1: big conv + dm + masks ----------------
            with (
                tc.tile_pool(name="p1", bufs=1) as p1pool,
                tc.tile_pool(name="p1y", bufs=4) as ypool,
                tc.tile_pool(name="p1s", bufs=3) as spool,
                tc.tile_pool(name="ps1", bufs=4, space="PSUM") as ps1pool,
                tc.tile_pool(name="psdm", bufs=2, space="PSUM") as psdmpool,
            ):
                xh_sb = p1pool.tile([128, 2 * NPPIX], F32R, tag="xh", name="xh_sb")
                nc.sync.dma_start(xh_sb[:], ins["xh_pad"][:])
                xh_v = xh_sb[:].rearrange("p (c h w) -> p c h w", c=2, h=HP, w=HP)
                wbig_v = wt["Wbig"][:].rearrange(
                    "p (t i o c) -> p t i o c", t=9, i=2, o=2, c=128
                )
                w2_v = wt["W2e"][:].rearrange("p (i m) -> p i m", i=2)

                for it, y0, cols in tiles():
                    att_t = spool.tile([9, NT], F32R, tag="att", name=f"att{it}")
                    nc.sync.dma_start(att_t[:], ins["att_shift"][:, cols])
                    att1_t = spool.tile([1, NT], F32, tag="att1", name=f"att1{it}")
                    nc.sync.dma_start(att1_t[:], ins["att_img"][:, cols])

                    dm_ps = psdmpool.tile([33, NT], F32, tag="dmps", name=f"dmps{it}")
                    for oh in range(2):
                        ps = ps1pool.tile([128, NT], F32, tag="bigps", name=f"bps{it}_{oh}")
                        first = True
                        for t in range(9):
                            ty, tx = t // 3, t % 3
                            for ih in range(2):
                                nc.tensor.matmul(
                                    ps[:],
                                    (wbig_v[:, t, ih, oh, :]),
                                    (xh_v[:, ih, y0 + ty : y0 + ty + ROWS_PER_TILE, tx : tx + W]),
                                    start=first,
                                    stop=False,
                                )
                                first = False
                        nc.tensor.matmul(
                            ps[:],
                            (wt["Watt"][:, oh * 128 : (oh + 1) * 128]),
                            (att_t[:]),
                            start=False,
                            stop=True,
                        )
                        y_t = ypool.tile([128, NT], F32R, tag="y", name=f"y{it}_{oh}")
                        nc.scalar.activation(
                            y_t[:], ps[:], RELU, bias=wt["bias_big"][:, oh : oh + 1]
                        )
                        nc.tensor.matmul(
                            dm_ps[:],
                            (w2_v[:, oh, :]),
                            (y_t[:]),
                            start=(oh == 0),
                            stop=(oh == 1),
                        )
                    # dm = conv + b2 -> DRAM output
                    dm_t = spool.tile([2, NT], F32, tag="dmt", name=f"dmt{it}")
                    nc.vector.tensor_scalar_add(dm_t[:], dm_ps[0:2, :], wt["bias_dm"][:, 0:1])
                    nc.sync.dma_start(out_dm[:, cols], dm_t[:])
                    # datt0 = sigmoid(dm0-dm1+bsd); m0 = datt0*att; m1 = att-m0
                    da_t = spool.tile([1, NT], F32, tag="dat", name=f"dat{it}")
                    nc.scalar.activation(
                        da_t[:], dm_ps[32:33, :], SIG, bias=wt["bias_sd"][:, 0:1]
                    )
                    m0_t = spool.tile([1, NT], F32, tag="m0", name=f"m0{it}")
                    nc.vector.tensor_mul(m0_t[:], da_t[:], att1_t[:])
                    m1_t = spool.tile([1, NT], F32, tag="m1", name=f"m1{it}")
                    nc.vector.tensor_sub(m1_t[:], att1_t[:], m0_t[:])
                    nc.sync.dma_start(m0_d[:, cols], m0_t[:])
                    nc.sync.dma_start(m1_d[:, cols], m1_t[:])

            # ---------------- Phase 2: branches + GRUs ----------------
            with (
                tc.tile_pool(name="p2", bufs=1) as p2pool,
                tc.tile_pool(name="p2b", bufs=3) as bpool,
                tc.tile_pool(name="ps2", bufs=6, space="PSUM") as ps2pool,
            ):
                pad_all = p2pool.tile([P_ROWS, NPPIX], F32R, tag="pad", name="pad_all")
                nc.vector.memset(pad_all[:].bitcast(F32), 0.0)
                pad_v = pad_all[:].rearrange("p (h w) -> p h w", h=HP, w=HP)
                x12_sb = p2pool.tile([20, NPIX], F32R, tag="x12", name="x12_sb")

                # DMA-written pad blocks
                nc.sync.dma_start(
                    pad_v[P_H1 : P_H1 + 10, 1:97, 1:97],
                    ins["h_nodes"][10:20, :].rearrange("p (h w) -> p h w", h=H, w=W),
                )
                nc.sync.dma_start(
                    pad_v[P_H2 : P_H2 + 10, 1:97, 1:97],
                    ins["h_nodes"][20:30, :].rearrange("p (h w) -> p h w", h=H, w=W),
                )
                nc.sync.dma_start(
                    pad_v[P_UP : P_UP + 10, 1:97, 1:97],
                    ins["upP"][:].rearrange("p (h w) -> p h w", h=H, w=W),
                )
                nc.sync.dma_start(
                    pad_v[P_LO : P_LO + 10, 1:97, 1:97],
                    ins["loP"][:].rearrange("p (h w) -> p h w", h=H, w=W),
                )

                # P0 = parent*m0 (base 0), P1 = parent*m1 (base 32), per tile
                for it, y0, cols in tiles():
                    r = ROWS_PER_TILE
                    ptile = bpool.tile([10, NT], F32, tag="ptile", name=f"pt{it}")
                    nc.sync.dma_start(ptile[:], ins["parent"][:, cols])
                    m0b = bpool.tile([10, NT], F32, tag="m0b", name=f"m0b{it}")
                    nc.sync.dma_start(m0b[:], m0_d[0:1, cols].broadcast_to([10, NT]))
                    m1b = bpool.tile([10, NT], F32, tag="m1b", name=f"m1b{it}")
                    nc.sync.dma_start(m1b[:], m1_d[0:1, cols].broadcast_to([10, NT]))
                    nc.vector.tensor_mul(
                        pad_v[P_P0 : P_P0 + 10, 1 + y0 : 1 + y0 + r, 1:97],
                        ptile[:].rearrange("p (h w) -> p h w", h=r, w=W),
                        m0b[:].rearrange("p (h w) -> p h w", h=r, w=W),
                    )
                    nc.vector.tensor_mul(
                        pad_v[P_P1 : P_P1 + 10, 1 + y0 : 1 + y0 + r, 1:97],
                        ptile[:].rearrange("p (h w) -> p h w", h=r, w=W),
                        m1b[:].rearrange("p (h w) -> p h w", h=r, w=W),
                    )

                # branch convs -> x12_sb
                l3_v = wt["L3"][:].rearrange("p (t m) -> p t m", t=9)
                for it, y0, cols in tiles():
                    ps3 = ps2pool.tile([40, NT], F32, tag="ps2", name=f"ps3_{it}")
                    for t in range(9):
                        ty, tx = t // 3, t % 3
                        nc.tensor.matmul(
                            ps3[:],
                            (l3_v[:, t, :]),
                            (pad_v[0:P_ROWS, y0 + ty : y0 + ty + ROWS_PER_TILE, tx : tx + W]),
                            start=(t == 0),
                            stop=(t == 8),
                        )
                    y3 = bpool.tile([40, NT], F32R, tag="y3", name=f"y3_{it}")
                    nc.scalar.activation(y3[:], ps3[:], RELU, bias=wt["bias3"][:, 0:1])
                    ps1x = ps2pool.tile([52, NT], F32, tag="ps2", name=f"ps1x_{it}")
                    nc.tensor.matmul(ps1x[:], (wt["L1"][:]), (y3[:]), start=True, stop=True)
                    bf_d = bpool.tile([20, NT], F32, tag="bfd", name=f"bfd{it}")
                    nc.scalar.activation(bf_d[:], ps1x[0:20, :], RELU, bias=wt["bias1x1_d"][:, 0:1])
                    bf_c = bpool.tile([20, NT], F32, tag="bfc", name=f"bfc{it}")
                    nc.scalar.activation(bf_c[:], ps1x[32:52, :], RELU, bias=wt["bias1x1_c"][:, 0:1])
                    # x1 = cu + d0 ; x2 = cl + d1
                    nc.vector.tensor_add(x12_sb[:, cols], bf_c[:], bf_d[:])

                # gates -> ru_d
                for it, y0, cols in tiles():
                    x0_t = bpool.tile([10, NT], F32R, tag="x0t", name=f"x0t{it}")
                    nc.sync.dma_start(x0_t[:], ins["x0"][:, cols])
                    h_t = bpool.tile([30, NT], F32R, tag="hg", name=f"hg{it}")
                    nc.sync.dma_start(h_t[:], ins["h_nodes"][:, cols])
                    psg = ps2pool.tile([6, NT], F32, tag="ps2", name=f"psg{it}")
                    nc.tensor.matmul(psg[:], (wt["Lg_x0"][:]), (x0_t[:]), start=True, stop=False)
                    nc.tensor.matmul(psg[:], (wt["Lg_x12"][:]), (x12_sb[:, cols]), start=False, stop=False)
                    nc.tensor.matmul(psg[:], (wt["Lg_h"][:]), (h_t[:]), start=False, stop=True)
                    ru_t = bpool.tile([6, NT], F32, tag="rut", name=f"rut{it}")
                    nc.scalar.activation(ru_t[:], psg[:], SIG, bias=wt["bias_g"][:, 0:1])
                    nc.sync.dma_start(ru_d[:, cols], ru_t[:])

                # cnm + gru update -> out_nodes
                for it, y0, cols in tiles():
                    x0_t = bpool.tile([10, NT], F32R, tag="x0t2", name=f"x0u{it}")
                    nc.sync.dma_start(x0_t[:], ins["x0"][:, cols])
                    h_t = bpool.tile([30, NT], F32R, tag="hu", name=f"hu{it}")
                    nc.sync.dma_start(h_t[:], ins["h_nodes"][:, cols])
                    rb_t = bpool.tile([30, NT], F32, tag="rbt", name=f"rbt{it}")
                    nc.sync.dma_start(
                        rb_t[:],
                        ru_d[0:3, cols].unsqueeze(1).broadcast_to([3, 10, NT]),
                    )
                    ub_t = bpool.tile([30, NT], F32, tag="ubt", name=f"ubt{it}")
                    nc.sync.dma_start(
                        ub_t[:],
                        ru_d[3:6, cols].unsqueeze(1).broadcast_to([3, 10, NT]),
                    )
                    rh_t = bpool.tile([30, NT], F32R, tag="rht", name=f"rht{it}")
                    nc.vector.tensor_mul(rh_t[:], h_t[:].bitcast(F32), rb_t[:])
                    psc = ps2pool.tile([30, NT], F32, tag="ps2", name=f"psc{it}")
                    nc.tensor.matmul(psc[:], (wt["Lc_x0"][:]), (x0_t[:]), start=True, stop=False)
                    nc.tensor.matmul(psc[:], (wt["Lc_x12"][:]), (x12_sb[:, cols]), start=False, stop=False)
                    nc.tensor.matmul(psc[:], (wt["Lc_rh"][:]), (rh_t[:]), start=False, stop=True)
                    cnm_t = bpool.tile([30, NT], F32, tag="cnmt", name=f"cnm{it}")
                    nc.scalar.activation(cnm_t[:], psc[:], RELU, bias=wt["bias_c"][:, 0:1])
                    # n = h + u*(cnm - h)
                    nc.vector.tensor_sub(cnm_t[:], cnm_t[:], h_t[:].bitcast(F32))
                    nc.vector.tensor_mul(cnm_t[:], cnm_t[:], ub_t[:])
                    n_t = bpool.tile([30, NT], F32, tag="nt", name=f"nt{it}")
                    nc.vector.tensor_add(n_t[:], h_t[:].bitcast(F32), cnm_t[:])
                    nc.sync.dma_start(out_nodes[:, cols], n_t[:])

    nc.compile()
    return nc


_NC_CACHE = [None]


def get_nc():
    if _NC_CACHE[0] is None:
        _NC_CACHE[0] = build_nc()
    return _NC_CACHE[0]


def make_in_maps(inputs):
    inputs = {k: np.asarray(v) for k, v in inputs.items()}
    w = pack_weights(inputs)
    in_maps = []
    for b in range(B):
        m = dict(w)
        m.update(pack_percore(inputs, b))
        in_maps.append(m)
    return in_maps


def assemble(results):
    nodes = np.stack(
        [results[b]["out_nodes"].reshape(3, HID, H, W) for b in range(B)], axis=1
    )
    dm = np.stack([results[b]["out_dm"].reshape(2, H, W) for b in range(B)], axis=0)
    return nodes, dm


def kernel(**inputs):
    nc = get_nc()
    in_maps = make_in_maps(inputs)
    res = run_bass_kernel_spmd(nc, in_maps, list(range(N_CORES)))
    return assemble(res.results)


# revision 10
# speedup vs baseline: 1.2781x; 1.2781x over previous
"""Trainium2 Bass kernel for nn_Half_Graph (GNN message passing half-graph).

Pure data-parallel over batch B=8 -> 8 NeuronCores, one sample per core.
Weights replicated; per-sample tensors sharded by batch index.

Outputs match reference: (stack([n0,n1,n2]) [3,8,10,96,96], dm [8,2,96,96]).

Constraints honored:
- compute-engine SBUF/PSUM operands start at partition 0/32/64/96
- DMA APs max 3 dims; DRAM-side APs may use 0-strides (broadcasts)
- fp32 matmuls as float32r (full rate for N >= 256); operands must be
  produced as float32r end-to-end
- one HW DMA queue per issuing engine (~25 GB/s each): sync + scalar HW
  queues + gpsimd SW queue, loads spread across them, xh band-pipelined
"""

import numpy as np

import concourse.bass as bass
import concourse.tile as tile
from concourse import bacc, mybir
from concourse.bass_utils import run_bass_kernel_spmd

F32 = mybir.dt.float32
F32R = mybir.dt.float32r

N_CORES = 8
B = 8
HID = 10
H = W = 96
HP = 98  # padded
NPIX = H * W  # 9216
NPPIX = HP * HP  # 9604

NT = 384  # matmul sub-tile columns (4 image rows)
SUB_ROWS = 4
G_ROWS = 16  # image rows per group
NG = H // G_ROWS  # 6 groups
NSUB = G_ROWS // SUB_ROWS  # 4 sub-tiles per group
GPIX = G_ROWS * W  # 1536 pixels per group
BAND = G_ROWS + 2  # padded rows per xh band

# pad_all row map ([84, 9604]); all compute writes at aligned bases
P_P0 = 0   # parent*m0 (0-9), parent*m1 (10-19)  -- one DVE mul at base 0
P_H1 = 32  # h1 (32-41), h2 (42-51)              -- one DVE copy at base 32
P_UP = 64  # up (64-73), lo (74-83)              -- one DVE copy at base 64
P_ROWS = 84


def _taps():
    return [(ty, tx) for ty in range(3) for tx in range(3)]


# --------------------------------------------------------------------------
# Host-side packing
# --------------------------------------------------------------------------


def pack_weights(inp):
    w = {}
    f32 = np.float32

    W1f = (inp["da_w1"] * inp["da_g1"][:, None, None, None]).astype(f32)  # (256,257,3,3)
    Wbig = np.zeros((128, 9, 2, 2, 128), f32)  # [ci, tap, cih, coh, co]
    for t, (ty, tx) in enumerate(_taps()):
        for ih in range(2):
            for oh in range(2):
                Wbig[:, t, ih, oh, :] = W1f[
                    oh * 128 : (oh + 1) * 128, 1 + ih * 128 : 1 + (ih + 1) * 128, ty, tx
                ].T
    w["Wbig"] = np.ascontiguousarray(Wbig.reshape(128, 9 * 2 * 2 * 128))
    Watt = np.zeros((9, 256), f32)
    for t, (ty, tx) in enumerate(_taps()):
        Watt[t, :] = W1f[:, 0, ty, tx]
    w["Watt"] = Watt
    w["bias_big"] = np.stack([inp["da_b1"][:128], inp["da_b1"][128:]], axis=1).astype(f32)

    # dm conv cols: 0=dm0, 1=dm1, 32=dm0-dm1 (PSUM base-32), rest zero
    W2f = inp["da_w2"][:, :, 0, 0].astype(f32)  # (2,256)
    W2e = np.zeros((128, 2, 33), f32)
    for ih in range(2):
        W2e[:, ih, 0] = W2f[0, ih * 128 : (ih + 1) * 128]
        W2e[:, ih, 1] = W2f[1, ih * 128 : (ih + 1) * 128]
        W2e[:, ih, 32] = W2e[:, ih, 0] - W2e[:, ih, 1]
    w["W2e"] = np.ascontiguousarray(W2e.reshape(128, 66))
    w["bias_dm"] = inp["da_b2"].reshape(2, 1).astype(f32)
    w["bias_sd"] = np.array([[inp["da_b2"][0] - inp["da_b2"][1]]], f32)

    # branch 3x3 convs, block-sparse over pad_all rows
    relW1 = (inp["rel_w1"] * inp["rel_g1"][:, None, None, None]).astype(f32)
    cuW1 = (inp["cu_w1"] * inp["cu_g1"][:, None, None, None]).astype(f32)
    clW1 = (inp["cl_w1"] * inp["cl_g1"][:, None, None, None]).astype(f32)
    L3 = np.zeros((P_ROWS, 9, 40), f32)
    for t, (ty, tx) in enumerate(_taps()):
        # out cols: d0 0-9, d1 10-19, cu 20-29, cl 30-39
        L3[0:10, t, 0:10] = relW1[:, 0:10, ty, tx].T        # P0 -> d0
        L3[32:42, t, 0:10] = relW1[:, 10:20, ty, tx].T      # h1 -> d0
        L3[10:20, t, 10:20] = relW1[:, 0:10, ty, tx].T      # P1 -> d1
        L3[42:52, t, 10:20] = relW1[:, 10:20, ty, tx].T     # h2 -> d1
        L3[32:42, t, 20:30] = cuW1[:, 0:10, ty, tx].T       # h1 -> cu
        L3[64:74, t, 20:30] = cuW1[:, 10:20, ty, tx].T      # up -> cu
        L3[42:52, t, 30:40] = clW1[:, 0:10, ty, tx].T       # h2 -> cl
        L3[74:84, t, 30:40] = clW1[:, 10:20, ty, tx].T      # lo -> cl
    w["L3"] = np.ascontiguousarray(L3.reshape(P_ROWS, 9 * 40))
    w["bias3"] = np.concatenate(
        [inp["rel_b1"], inp["rel_b1"], inp["cu_b1"], inp["cl_b1"]]
    ).reshape(40, 1).astype(f32)

    # branch 1x1 convs: out cols d0 0-9, d1 10-19, cu 32-41, cl 42-51
    relW2 = (inp["rel_w2"][:, :, 0, 0] * inp["rel_g2"][:, None]).astype(f32)
    cuW2 = (inp["cu_w2"][:, :, 0, 0] * inp["cu_g2"][:, None]).astype(f32)
    clW2 = (inp["cl_w2"][:, :, 0, 0] * inp["cl_g2"][:, None]).astype(f32)
    L1 = np.zeros((40, 52), f32)
    L1[0:10, 0:10] = relW2.T
    L1[10:20, 10:20] = relW2.T
    L1[20:30, 32:42] = cuW2.T
    L1[30:40, 42:52] = clW2.T
    w["L1"] = L1
    w["bias1x1_d"] = np.concatenate([inp["rel_b2"], inp["rel_b2"]]).reshape(20, 1).astype(f32)
    w["bias1x1_c"] = np.concatenate([inp["cu_b2"], inp["cl_b2"]]).reshape(20, 1).astype(f32)

    # GRU gates split-K; psum rows r0 r1 r2 u0 u1 u2
    Lg_x0 = np.zeros((10, 6), f32)
    Lg_x12 = np.zeros((20, 6), f32)
    Lg_h = np.zeros((30, 6), f32)
    for k in range(3):
        gw = inp["gru_gw"][k][:, :, 0, 0].astype(f32)  # (2, 20)
        if k == 0:
            Lg_x0[:, 0] = gw[0, 0:10]
            Lg_x0[:, 3] = gw[1, 0:10]
        else:
            Lg_x12[(k - 1) * 10 : k * 10, k] = gw[0, 0:10]
            Lg_x12[(k - 1) * 10 : k * 10, 3 + k] = gw[1, 0:10]
        Lg_h[k * 10 : (k + 1) * 10, k] = gw[0, 10:20]
        Lg_h[k * 10 : (k + 1) * 10, 3 + k] = gw[1, 10:20]
    w["Lg_x0"] = Lg_x0
    w["Lg_x12"] = Lg_x12
    w["Lg_h"] = Lg_h
    w["bias_g"] = np.concatenate(
        [inp["gru_gb"][:, 0], inp["gru_gb"][:, 1]]
    ).reshape(6, 1).astype(f32)

    # GRU cnm split-K
    Lc_x0 = np.zeros((10, 30), f32)
    Lc_x12 = np.zeros((20, 30), f32)
    Lc_rh = np.zeros((30, 30), f32)
    for k in range(3):
        cw = (inp["gru_cw"][k][:, :, 0, 0] * inp["gru_cg"][k][:, None]).astype(f32)  # (10,20)
        if k == 0:
            Lc_x0[:, 0:10] = cw[:, 0:10].T
        else:
            Lc_x12[(k - 1) * 10 : k * 10, k * 10 : (k + 1) * 10] = cw[:, 0:10].T
        Lc_rh[k * 10 : (k + 1) * 10, k * 10 : (k + 1) * 10] = cw[:, 10:20].T
    w["Lc_x0"] = Lc_x0
    w["Lc_x12"] = Lc_x12
    w["Lc_rh"] = Lc_rh
    w["bias_c"] = inp["gru_cb"].reshape(30, 1).astype(f32)

    # r/u partition-broadcast matmul: rb cols 0-29, ub cols 32-61
    Eru = np.zeros((6, 62), f32)
    for k in range(3):
        Eru[k, k * 10 : (k + 1) * 10] = 1.0
        Eru[3 + k, 32 + k * 10 : 42 + k * 10] = 1.0
    w["Eru"] = Eru
    return w


def pack_percore(inp, b):
    f32 = np.float32
    d = {}
    # padded xh: [ci(128), cih(2), 98, 98]
    xh = inp["xh"][b].reshape(2, 128, 96, 96)
    xp = np.zeros((2, 128, HP, HP), f32)
    xp[:, :, 1:97, 1:97] = xh
    d["xh_pad"] = np.ascontiguousarray(xp.transpose(1, 0, 2, 3).reshape(128, 2 * NPPIX))
    # shifted parent-attention copies (one per conv tap)
    ap = np.zeros((HP, HP), f32)
    ap[1:97, 1:97] = inp["f_atts"][1, b, 0]
    ash = np.empty((9, NPIX), f32)
    for t, (ty, tx) in enumerate(_taps()):
        ash[t] = ap[ty : ty + 96, tx : tx + 96].reshape(-1)
    d["att_shift"] = ash
    d["att_img"] = np.ascontiguousarray(inp["f_atts"][1, b, 0].reshape(1, NPIX))
    # input-only sums / products (host precompute)
    d["x0"] = (inp["f_nodes"][0, b] + inp["p_nodes"][0, b]).reshape(10, NPIX).astype(f32)
    up = (inp["p_nodes"][1:5, b].sum(0) * inp["p_atts"][1:5, b].sum(0)).reshape(10, NPIX)
    lo = (inp["p_nodes"][5:7, b].sum(0) * inp["p_atts"][5:7, b].sum(0)).reshape(10, NPIX)
    d["uplo"] = np.concatenate([up, lo], axis=0).astype(f32)
    d["parent"] = np.ascontiguousarray(inp["f_nodes"][1, b].reshape(10, NPIX))
    d["h_nodes"] = np.ascontiguousarray(inp["h_nodes"][:, b].reshape(30, NPIX))
    d["h12"] = np.ascontiguousarray(inp["h_nodes"][1:3, b].reshape(20, NPIX))
    return d


# --------------------------------------------------------------------------
# Device kernel
# --------------------------------------------------------------------------

WEIGHT_SPECS = [
    ("Wbig", (128, 4608)),
    ("Watt", (9, 256)),
    ("bias_big", (128, 2)),
    ("W2e", (128, 66)),
    ("bias_dm", (2, 1)),
    ("bias_sd", (1, 1)),
    ("L3", (P_ROWS, 360)),
    ("bias3", (40, 1)),
    ("L1", (40, 52)),
    ("bias1x1_d", (20, 1)),
    ("bias1x1_c", (20, 1)),
    ("Lg_x0", (10, 6)),
    ("Lg_x12", (20, 6)),
    ("Lg_h", (30, 6)),
    ("bias_g", (6, 1)),
    ("Lc_x0", (10, 30)),
    ("Lc_x12", (20, 30)),
    ("Lc_rh", (30, 30)),
    ("bias_c", (30, 1)),
    ("Eru", (6, 62)),
]

PERCORE_SPECS = [
    ("xh_pad", (128, 2 * NPPIX)),
    ("att_shift", (9, NPIX)),
    ("att_img", (1, NPIX)),
    ("x0", (10, NPIX)),
    ("uplo", (20, NPIX)),
    ("parent", (10, NPIX)),
    ("h_nodes", (30, NPIX)),
    ("h12", (20, NPIX)),
]

# tensors feeding matmuls are float32r end-to-end (same bits as f32)
R_NAMES = {
    "Wbig", "Watt", "W2e", "L3", "L1", "Lg_x0", "Lg_x12", "Lg_h",
    "Lc_x0", "Lc_x12", "Lc_rh", "Eru",
    "xh_pad", "att_shift", "x0", "uplo", "h_nodes", "h12",
}


def dt_of(name):
    return F32R if name in R_NAMES else F32


def build_nc():
    nc = bacc.Bacc("TRN2", target_bir_lowering=False, debug=False, num_devices=N_CORES)

    ins = {}
    for name, shape in WEIGHT_SPECS + PERCORE_SPECS:
        ins[name] = nc.dram_tensor(name, list(shape), dt_of(name), kind="ExternalInput").ap()
    out_nodes = nc.dram_tensor("out_nodes", [30, NPIX], F32, kind="ExternalOutput").ap()
    out_dm = nc.dram_tensor("out_dm", [2, NPIX], F32, kind="ExternalOutput").ap()

    RELU = mybir.ActivationFunctionType.Relu
    SIG = mybir.ActivationFunctionType.Sigmoid

    with tile.TileContext(nc) as tc:
        with (
            tc.tile_pool(name="wpool", bufs=1) as wpool,
            tc.tile_pool(name="drampool", bufs=1, space="DRAM") as drampool,
        ):
            wt = {}
            for name, shape in WEIGHT_SPECS:
                wt[name] = wpool.tile(list(shape), dt_of(name), tag=name, name="wt_" + name)
                nc.gpsimd.dma_start(wt[name][:], ins[name][:])

            m01_d = drampool.tile([2, NPIX], F32, name="m01_d")

            # ---------------- Phase 1: big conv + dm + masks ----------------
            with (
                tc.tile_pool(name="p1band", bufs=2) as bandpool,
                tc.tile_pool(name="p1y", bufs=4) as ypool,
                tc.tile_pool(name="p1g", bufs=2) as gpool,
                tc.tile_pool(name="ps1", bufs=4, space="PSUM") as ps1pool,
                tc.tile_pool(name="psdm", bufs=2, space="PSUM") as psdmpool,
            ):
                xh_dram = ins["xh_pad"][:].rearrange("p (c h w) -> p c h w", c=2, h=HP, w=HP)
                wbig_v = wt["Wbig"][:].rearrange(
                    "p (t i o c) -> p t i o c", t=9, i=2, o=2, c=128
                )
                w2_v = wt["W2e"][:].rearrange("p (i m) -> p i m", i=2)

                for g in range(NG):
                    gy = g * G_ROWS
                    gcols = slice(gy * W, (gy + G_ROWS) * W)
                    # xh band: padded rows gy .. gy+17, halves on separate queues
                    xh_g = bandpool.tile([128, 2 * BAND * HP], F32R, tag="band", name=f"band{g}")
                    xh_gv = xh_g[:].rearrange("p (c h w) -> p c h w", c=2, h=BAND, w=HP)
                    nc.sync.dma_start(xh_gv[:, 0, :, :], xh_dram[:, 0, gy : gy + BAND, :])
                    nc.scalar.dma_start(xh_gv[:, 1, :, :], xh_dram[:, 1, gy : gy + BAND, :])

                    att_g = gpool.tile([9, GPIX], F32R, tag="attg", name=f"attg{g}")
                    nc.gpsimd.dma_start(att_g[:], ins["att_shift"][:, gcols])
                    att1_g = gpool.tile([1, GPIX], F32, tag="att1g", name=f"att1g{g}")
                    nc.gpsimd.dma_start(att1_g[:], ins["att_img"][:, gcols])

                    dm_g = gpool.tile([2, GPIX], F32, tag="dmg", name=f"dmg{g}")
                    da_g = gpool.tile([1, GPIX], F32, tag="dag", name=f"dag{g}")

                    for s in range(NSUB):
                        ly = SUB_ROWS * s
                        sub = slice(s * NT, (s + 1) * NT)
                        dm_ps = psdmpool.tile([33, NT], F32, tag="dmps", name=f"dmps{g}_{s}")
                        for oh in range(2):
                            ps = ps1pool.tile([128, NT], F32, tag="bigps", name=f"bps{g}{s}{oh}")
                            first = True
                            for t in range(9):
                                ty, tx = t // 3, t % 3
                                nc.tensor.matmul(
                                    ps[:],
                                    wbig_v[:, t, 0, oh, :],
                                    xh_gv[:, 0, ly + ty : ly + ty + SUB_ROWS, tx : tx + W],
                                    start=first, stop=False,
                                )
                                first = False
                                nc.tensor.matmul(
                                    ps[:],
                                    wbig_v[:, t, 1, oh, :],
                                    xh_gv[:, 1, ly + ty : ly + ty + SUB_ROWS, tx : tx + W],
                                    start=False, stop=False,
                                )
                            nc.tensor.matmul(
                                ps[:],
                                wt["Watt"][:, oh * 128 : (oh + 1) * 128],
                                att_g[:, sub],
                                start=False, stop=True,
                            )
                            y_t = ypool.tile([128, NT], F32R, tag="y", name=f"y{g}{s}{oh}")
                            nc.scalar.activation(
                                y_t[:], ps[:], RELU, bias=wt["bias_big"][:, oh : oh + 1]
                            )
                            nc.tensor.matmul(
                                dm_ps[:], w2_v[:, oh, :], y_t[:],
                                start=(oh == 0), stop=(oh == 1),
                            )
                        nc.vector.tensor_scalar_add(
                            dm_g[:, sub], dm_ps[0:2, :], wt["bias_dm"][:, 0:1]
                        )
                        nc.scalar.activation(
                            da_g[:, sub], dm_ps[32:33, :], SIG, bias=wt["bias_sd"][:, 0:1]
                        )
                    # masks for the whole group
                    m0_g = gpool.tile([1, GPIX], F32, tag="m0g", name=f"m0g{g}")
                    nc.vector.tensor_mul(m0_g[:], da_g[:], att1_g[:])
                    m1_g = gpool.tile([1, GPIX], F32, tag="m1g", name=f"m1g{g}")
                    nc.vector.tensor_sub(m1_g[:], att1_g[:], m0_g[:])
                    nc.gpsimd.dma_start(out_dm[:, gcols], dm_g[:])
                    nc.gpsimd.dma_start(m01_d[0:1, gcols], m0_g[:])
                    nc.gpsimd.dma_start(m01_d[1:2, gcols], m1_g[:])

            # ---------------- Phase 2: branches + GRUs ----------------
            with (
                tc.tile_pool(name="p2", bufs=1) as p2pool,
                tc.tile_pool(name="p2g", bufs=3) as g2pool,
                tc.tile_pool(name="p2s", bufs=3) as s2pool,
                tc.tile_pool(name="ps2", bufs=6, space="PSUM") as ps2pool,
            ):
                pad_all = p2pool.tile([P_ROWS, NPPIX], F32R, tag="pad", name="pad_all")
                nc.vector.memset(pad_all[:].bitcast(F32), 0.0)
                pad_v = pad_all[:].rearrange("p (h w) -> p h w", h=HP, w=HP)
                x12_sb = p2pool.tile([20, NPIX], F32R, tag="x12", name="x12_sb")

                # build pad interiors, group at a time
                for g in range(NG):
                    gy = g * G_ROWS
                    gcols = slice(gy * W, (gy + G_ROWS) * W)
                    rows = slice(1 + gy, 1 + gy + G_ROWS)
                    # parent twice [20, GPIX] via DRAM 0-stride
                    pt2 = g2pool.tile([20, GPIX], F32, tag="g20", name=f"pt2_{g}")
                    nc.scalar.dma_start(
                        pt2[:],
                        ins["parent"][:, gcols].unsqueeze(0).broadcast_to([2, 10, GPIX]),
                    )
                    # m0 rows 0-9, m1 rows 10-19
                    m01b = g2pool.tile([20, GPIX], F32, tag="g20", name=f"m01b{g}")
                    nc.sync.dma_start(
                        m01b[:],
                        m01_d[:, gcols].unsqueeze(1).broadcast_to([2, 10, GPIX]),
                    )
                    nc.vector.tensor_mul(
                        pad_v[0:20, rows, 1:97],
                        pt2[:].rearrange("p (h w) -> p h w", h=G_ROWS, w=W),
                        m01b[:].rearrange("p (h w) -> p h w", h=G_ROWS, w=W),
                    )
                    h12_g = g2pool.tile([20, GPIX], F32R, tag="g20", name=f"h12g{g}")
                    nc.scalar.dma_start(h12_g[:], ins["h12"][:, gcols])
                    nc.vector.tensor_copy(
                        pad_v[32:52, rows, 1:97],
                        h12_g[:].rearrange("p (h w) -> p h w", h=G_ROWS, w=W),
                    )
                    uplo_g = g2pool.tile([20, GPIX], F32R, tag="g20", name=f"uplog{g}")
                    nc.sync.dma_start(uplo_g[:], ins["uplo"][:, gcols])
                    nc.vector.tensor_copy(
                        pad_v[64:84, rows, 1:97],
                        uplo_g[:].rearrange("p (h w) -> p h w", h=G_ROWS, w=W),
                    )

                # branch convs -> x12_sb
                l3_v = wt["L3"][:].rearrange("p (t m) -> p t m", t=9)
                for g in range(NG):
                    gy = g * G_ROWS
                    for s in range(NSUB):
                        y0 = gy + SUB_ROWS * s
                        cols = slice(y0 * W, (y0 + SUB_ROWS) * W)
                        ps3 = ps2pool.tile([40, NT], F32, tag="ps2", name=f"ps3_{g}{s}")
                        for t in range(9):
                            ty, tx = t // 3, t % 3
                            nc.tensor.matmul(
                                ps3[:],
                                l3_v[:, t, :],
                                pad_v[0:P_ROWS, y0 + ty : y0 + ty + SUB_ROWS, tx : tx + W],
                                start=(t == 0), stop=(t == 8),
                            )
                        y3 = s2pool.tile([40, NT], F32R, tag="y3", name=f"y3_{g}{s}")
                        nc.scalar.activation(y3[:], ps3[:], RELU, bias=wt["bias3"][:, 0:1])
                        ps1x = ps2pool.tile([52, NT], F32, tag="ps2", name=f"ps1x_{g}{s}")
                        nc.tensor.matmul(ps1x[:], wt["L1"][:], y3[:], start=True, stop=True)
                        bf_d = s2pool.tile([20, NT], F32, tag="bfd", name=f"bfd{g}{s}")
                        nc.scalar.activation(bf_d[:], ps1x[0:20, :], RELU, bias=wt["bias1x1_d"][:, 0:1])
                        bf_c = s2pool.tile([20, NT], F32, tag="bfc", name=f"bfc{g}{s}")
                        nc.scalar.activation(bf_c[:], ps1x[32:52, :], RELU, bias=wt["bias1x1_c"][:, 0:1])
                        nc.vector.tensor_add(x12_sb[:, cols], bf_c[:], bf_d[:])

                # gates + cnm + gru update, grouped loads
                for g in range(NG):
                    gy = g * G_ROWS
                    gcols = slice(gy * W, (gy + G_ROWS) * W)
                    x0_g = g2pool.tile([10, GPIX], F32R, tag="x0g", name=f"x0g{g}")
                    nc.sync.dma_start(x0_g[:], ins["x0"][:, gcols])
                    h_g = g2pool.tile([30, GPIX], F32R, tag="hg", name=f"hg{g}")
                    nc.scalar.dma_start(h_g[:], ins["h_nodes"][:, gcols])
                    n_g = g2pool.tile([30, GPIX], F32, tag="ng", name=f"ng{g}")
                    for s in range(NSUB):
                        y0 = gy + SUB_ROWS * s
                        cols = slice(y0 * W, (y0 + SUB_ROWS) * W)
                        sub = slice(s * NT, (s + 1) * NT)
                        psg = ps2pool.tile([6, NT], F32, tag="ps2", name=f"psg{g}{s}")
                        nc.tensor.matmul(psg[:], wt["Lg_x0"][:], x0_g[:, sub], start=True, stop=False)
                        nc.tensor.matmul(psg[:], wt["Lg_x12"][:], x12_sb[:, cols], start=False, stop=False)
                        nc.tensor.matmul(psg[:], wt["Lg_h"][:], h_g[:, sub], start=False, stop=True)
                        ru_t = s2pool.tile([6, NT], F32R, tag="rut", name=f"rut{g}{s}")
                        nc.scalar.activation(ru_t[:], psg[:], SIG, bias=wt["bias_g"][:, 0:1])
                        # broadcast r,u to 30 rows via matmul (rb 0-29, ub 32-61)
                        ps_ru = ps2pool.tile([62, NT], F32, tag="ps2", name=f"psru{g}{s}")
                        nc.tensor.matmul(ps_ru[:], wt["Eru"][:], ru_t[:], start=True, stop=True)
                        rh_t = s2pool.tile([30, NT], F32R, tag="rht", name=f"rht{g}{s}")
                        nc.vector.tensor_mul(rh_t[:], h_g[:, sub].bitcast(F32), ps_ru[0:30, :])
                        psc = ps2pool.tile([30, NT], F32, tag="ps2", name=f"psc{g}{s}")
                        nc.tensor.matmul(psc[:], wt["Lc_x0"][:], x0_g[:, sub], start=True, stop=False)
                        nc.tensor.matmul(psc[:], wt["Lc_x12"][:], x12_sb[:, cols], start=False, stop=False)
                        nc.tensor.matmul(psc[:], wt["Lc_rh"][:], rh_t[:], start=False, stop=True)
                        cnm_t = s2pool.tile([30, NT], F32, tag="cnmt", name=f"cnm{g}{s}")
                        nc.scalar.activation(cnm_t[:], psc[:], RELU, bias=wt["bias_c"][:, 0:1])
                        # n = h + u*(cnm - h)
                        nc.vector.tensor_sub(cnm_t[:], cnm_t[:], h_g[:, sub].bitcast(F32))
                        nc.vector.tensor_mul(cnm_t[:], cnm_t[:], ps_ru[32:62, :])
                        nc.vector.tensor_add(n_g[:, sub], h_g[:, sub].bitcast(F32), cnm_t[:])
                    nc.gpsimd.dma_start(out_nodes[:, gcols], n_g[:])

    nc.compile()
    return nc


_NC_CACHE = [None]


def get_nc():
    if _NC_CACHE[0] is None:
        _NC_CACHE[0] = build_nc()
    return _NC_CACHE[0]


def make_in_maps(inputs):
    inputs = {k: np.asarray(v) for k, v in inputs.items()}
    w = pack_weights(inputs)
    in_maps = []
    for b in range(B):
        m = dict(w)
        m.update(pack_percore(inputs, b))
        in_maps.append(m)
    return in_maps


def assemble(results):
    nodes = np.stack(
        [results[b]["out_nodes"].reshape(3, HID, H, W) for b in range(B)], axis=1
    )
    dm = np.stack([results[b]["out_dm"].reshape(2, H, W) for b in range(B)], axis=0)
    return nodes, dm


def kernel(**inputs):
    nc = get_nc()
    in_maps = make_in_maps(inputs)
    res = run_bass_kernel_spmd(nc, in_maps, list(range(N_CORES)))
    return assemble(res.results)
